# revision 22
# baseline (speedup 1.0000x reference)
"""Relative-position attention (Music-Transformer style skew) + LayerNorm,
distributed over 8 TRN2 NeuronCores.

Sharding: data-parallel over batch (B=4) x tensor-parallel over head-halves
(H=8 -> 2 groups of 4). Core c handles batch b=c//2, heads [4*(c%2), 4*(c%2)+4),
producing output channels [256*(c%2), +256) of y[b]. The final LayerNorm needs
full-E stats, exchanged via small pairwise AllReduces of (sum, sumsq).

Skew trick: Srel[i,j] = F[(i+1)*S + j] where F is the row-major flat view of
the padded matrix P[i, 0]=0, P[i, 1+l]=QEr[i, l] (P is [S, S+1]). We bounce P
through DRAM in bf16; the skewed read back is a plain strided DMA.

v3 structure (from trace analysis of the previous versions):
  - scoresT[t, q] = K Q^T + Srel^T is accumulated directly in PSUM: QK^T with
    kT as stationary (head pairs packed into the two 64-row PE halves),
    Srel^T via plain transpose-matmuls (srel stationary, identity moving,
    start=False) into the same banks.  No DVE adds, no separate score
    transposes; one wide exp() per t-block reads the 2-bank PSUM tile and
    produces the 2-head expT tile that AV consumes as stationary.
  - srel skew reads are SWDGE (gpsimd ring) and prefetched one (pair,qc)
    iteration ahead, so the PE never waits on them; P writes stay on the
    Sync HWDGE ring.
  - phase A (QEr -> P) is emitted in per-chunk micro-steps interleaved one
    per t-block, casts mostly on DVE, so neither ACT nor PSUM reuse stalls
    the PE.
  - LayerNorm stats/finalize are chunked: one small AllReduce per q-chunk
    issued as soon as that chunk's outputs are done, so only the last
    chunk's collective sits on the critical tail.
"""

from contextlib import ExitStack

import numpy as np

import concourse.bass as bass
import concourse.mybir as mybir
from concourse import masks
from concourse.tile import TileContext

F32 = mybir.dt.float32
F32R = mybir.dt.float32r
BF16 = mybir.dt.bfloat16
F8 = mybir.dt.float8e4

B, S, E, H = 4, 2048, 512, 8
HD = E // H          # 64
HLOC = 4             # heads per core
CH = HLOC * HD       # 256 output channels per core
SCALE = float(E) ** -0.5
EPS = 1e-5
N_CORES = 8


def build_nc(s=S, n_cores=N_CORES, debug=False, legalize=True):
    """Build the per-core Bass graph (SPMD: same graph on all cores)."""
    nc = bass.Bass(target_bir_lowering=False, debug=debug)

    SB = s // 128        # number of 128-row q/t blocks
    NTB = SB             # t blocks
    QPC = min(512, s)    # q columns per PSUM bank
    NQC = s // QPC       # q chunks
    QBC = QPC // 128     # q blocks per chunk
    CK = min(512, s)     # phase-A l chunk
    NCK = s // CK        # phase-A chunks per block

    x_d = nc.declare_dram_parameter("x", [s, E], F32, isOutput=False)
    wq_d = nc.declare_dram_parameter("wq", [CH, E], F32, isOutput=False)
    wk_d = nc.declare_dram_parameter("wk", [CH, E], F32, isOutput=False)
    wv_d = nc.declare_dram_parameter("wv", [CH, E], F32, isOutput=False)
    er_d = nc.declare_dram_parameter("er", [s, HD], F32, isOutput=False)
    gamma_d = nc.declare_dram_parameter("gamma", [1, CH], F32, isOutput=False)
    beta_d = nc.declare_dram_parameter("beta", [1, CH], F32, isOutput=False)
    out_d = nc.declare_dram_parameter("out", [s, CH], F32, isOutput=True)

    # Padded-QEr bounce buffers, one per head, flat [S*(S+1)] bf16.
    p_d = nc.dram_tensor("pbuf", [HLOC, s * (s + 1)], F8)

    # LayerNorm stat-exchange chunks: whole q-chunks early, then 2/1/1
    # blocks at the end so the final collective+LN covers only 128 rows.
    SB_ = s // 128
    if SB_ >= 8:
        chunks = [(0, SB_ // 2), (SB_ // 2, 3 * SB_ // 4),
                  (3 * SB_ // 4, SB_ - 1), (SB_ - 1, SB_)]
    elif SB_ >= 4:
        chunks = [(0, SB_ // 2), (SB_ // 2, SB_ - 1), (SB_ - 1, SB_)]
    else:
        chunks = [(0, SB_)]
    chunk_of = {}
    for ci, (cb0, cb1) in enumerate(chunks):
        for sb in range(cb0, cb1):
            chunk_of[sb] = ci
    cc_in_q = [nc.dram_tensor(f"cc_in{i}", [(c1 - c0) * 128, 2], F32)
               for i, (c0, c1) in enumerate(chunks)]
    cc_out_q = [nc.dram_tensor(f"cc_out{i}", [(c1 - c0) * 128, 2], F32)
                for i, (c0, c1) in enumerate(chunks)]

    pairs = [[2 * i, 2 * i + 1] for i in range(n_cores // 2)]

    with TileContext(nc) as tc:
        with (
            tc.tile_pool(name="const", bufs=1) as const_pool,
            tc.tile_pool(name="persist", bufs=1) as pp,
        ):
            ident_f32 = const_pool.tile([128, 128], F32)
            ident_f8 = const_pool.tile([128, 128], F8)
            masks.make_identity(nc, ident_f32[:])
            masks.make_identity(nc, ident_f8[:])
            gamma_bc = const_pool.tile([128, CH], F32)
            beta_bc = const_pool.tile([128, CH], F32)
            eps_t = const_pool.tile([128, 1], F32)
            nc.gpsimd.memset(eps_t[:], EPS)
            nc.sync.dma_start(gamma_bc[:], gamma_d[:].broadcast_to((128, CH)))
            nc.sync.dma_start(beta_bc[:], beta_d[:].broadcast_to((128, CH)))

            # ---- persistent SBUF tensors ----
            # ErT replicated into both partition halves so every head's
            # matmul finds it at its own base partition (PE requires
            # lhsT/rhs base partitions to match).
            # q/k/Er in bf16: fp32r matmuls stream at HALF rate on the PE
            # (two fp32_mode passes per column, seen in traces); bf16 cuts
            # the QK and QEr matmul time in half. The P bounce is bf16
            # anyway, so the precision loss is negligible at 2e-2 budget.
            erT = pp.tile([128, s], BF16, tag="erT")
            qT = [pp.tile([128, s], BF16, tag=f"qT{oc}", name=f"qT{oc}")
                  for oc in range(2)]
            kT = [pp.tile([128, s], BF16, tag=f"kT{oc}", name=f"kT{oc}")
                  for oc in range(2)]
            # v with a ones column appended per head: [128, HLOC*(HD+1)] bf16
            vaug = [pp.tile([128, HLOC * (HD + 1)], BF16, tag=f"va{sb}",
                            name=f"va{sb}") for sb in range(SB)]
            outp = [pp.tile([128, CH], F32, tag=f"op{sb}", name=f"op{sb}")
                    for sb in range(SB)]

            # ================= setup: load + transpose =================
            # xT / wT only live until the projections are done; scope them
            # so their SBUF is recycled for the attention pools.
            with tc.tile_pool(name="xw", bufs=1) as xw:
                xT = [xw.tile([128, s], BF16, tag=f"xT{ec}", name=f"xT{ec}")
                      for ec in range(4)]
                wT = {
                    w: [xw.tile([128, CH], BF16, tag=f"{w}T{ec}",
                                name=f"{w}T{ec}")
                        for ec in range(4)]
                    for w in ("wq", "wk", "wv")
                }
                setup_stack = ExitStack()
                ld_pool = setup_stack.enter_context(
                    tc.tile_pool(name="ld", bufs=4))
                ps_set = setup_stack.enter_context(
                    tc.tile_pool(name="ps_set", bufs=4, space="PSUM"))

                # Warm-up: absorb the Pool (identity-creation) dependency
                # into PE's observed clock.
                warm = ps_set.tile([128, 128], F32, tag="pset")
                nc.tensor.matmul(
                    warm[:], ident_f32[:], ident_f32[:], start=True, stop=True)

                # xT[ec][:, i*128:(i+1)*128] = x[i-block, ec-block].T
                # (plain matmuls against identity: cheaper than
                # transpose-mode and they count as PE activity for HAM)
                for sb in range(SB):
                    xt = ld_pool.tile([128, E], F32, tag="xld")
                    nc.sync.dma_start(xt[:], x_d[sb * 128:(sb + 1) * 128, :])
                    for ec in range(4):
                        pst = ps_set.tile([128, 128], F32, tag="pset")
                        nc.tensor.matmul(
                            pst[:], xt[:, ec * 128:(ec + 1) * 128],
                            ident_f32[:], start=True, stop=True)
                        nc.vector.tensor_copy(
                            xT[ec][:, sb * 128:(sb + 1) * 128], pst[:])
                # weights
                for w_name, w_d in (("wq", wq_d), ("wk", wk_d), ("wv", wv_d)):
                    for pc in range(CH // 128):
                        wt = ld_pool.tile([128, E], F32, tag="wld")
                        nc.sync.dma_start(
                            wt[:], w_d[pc * 128:(pc + 1) * 128, :])
                        for ec in range(4):
                            pst = ps_set.tile([128, 128], F32, tag="pset")
                            nc.tensor.matmul(
                                pst[:], wt[:, ec * 128:(ec + 1) * 128],
                                ident_f32[:], start=True, stop=True)
                            nc.vector.tensor_copy(
                                wT[w_name][ec][:, pc * 128:(pc + 1) * 128],
                                pst[:])
                # Er: transpose into both partition halves, one DVE copy
                for sb in range(SB):
                    et = ld_pool.tile([128, HD], F32, tag="eld")
                    nc.sync.dma_start(et[:], er_d[sb * 128:(sb + 1) * 128, :])
                    pst = ps_set.tile([128, 128], F32, tag="psete")
                    nc.tensor.matmul(
                        pst[0:64, :], et[:], ident_f32[:],
                        start=True, stop=True)
                    nc.tensor.matmul(
                        pst[64:128, :], et[:], ident_f32[:],
                        start=True, stop=True)
                    nc.vector.tensor_copy(
                        erT[:, sb * 128:(sb + 1) * 128], pst[:])

                # ================= projections =================
                setup_stack.close()
                with tc.tile_pool(
                        name="ps_pj", bufs=4, space="PSUM") as ps_pj:
                    # qT / kT: [oc*128+p, t] = sum_e W[oc*128+p, e] x[t, e]
                    for dst, w_name in ((qT, "wq"), (kT, "wk")):
                        for oc in range(2):
                            for sc in range(s // 512):
                                ps = ps_pj.tile([128, 512], F32, tag="pj")
                                for ec in range(4):
                                    nc.tensor.matmul(
                                        ps[:],
                                        wT[w_name][ec][:, oc * 128:
                                                       (oc + 1) * 128],
                                        xT[ec][:, sc * 512:(sc + 1) * 512],
                                        start=(ec == 0), stop=(ec == 3))
                                nc.scalar.copy(
                                    dst[oc][:, sc * 512:(sc + 1) * 512],
                                    ps[:])
                    # v natural + ones column, bf16
                    for sb in range(SB):
                        ps = ps_pj.tile([128, CH], F32, tag="pj")
                        for ec in range(4):
                            nc.tensor.matmul(
                                ps[:],
                                xT[ec][:, sb * 128:(sb + 1) * 128],
                                wT["wv"][ec][:],
                                start=(ec == 0), stop=(ec == 3))
                        for h in range(HLOC):
                            nc.scalar.copy(
                                vaug[sb][:, h * (HD + 1):h * (HD + 1) + HD],
                                ps[:, h * HD:(h + 1) * HD])
                            nc.vector.memset(
                                vaug[sb][:, h * (HD + 1) + HD:
                                         (h + 1) * (HD + 1)],
                                1.0)

            # ================= attention =================
            with (
                tc.tile_pool(name="wrk", bufs=2) as wrk,
                tc.tile_pool(name="srl", bufs=16) as srl,
                tc.tile_pool(name="exp", bufs=NTB + 2) as expp,
                tc.tile_pool(name="small", bufs=8) as small,
                tc.tile_pool(name="lnw", bufs=2) as lnw,
                # phase-A and AV accumulators share one 2-bank slot set
                # (tag "qa_av"): their uses are time-disjoint within an
                # iteration, and this frees 2 banks so the score tiles can
                # triple-buffer (exp latency never blocks the next QK).
                tc.tile_pool(name="ps_mix", bufs=4, space="PSUM") as ps_mix,
                tc.tile_pool(name="ps_sc", bufs=2, space="PSUM") as ps_sc,
            ):
                ps_qa = ps_av = ps_mix
                class PhaseA:
                    """QEr -> padded P rows for one head pair, emitted as
                    per-chunk micro-steps (2 packed matmuls + 2 casts each)
                    so the work interleaves into the score loop without
                    stalling PE or PSUM."""

                    def __init__(self, pr):
                        self.pr = pr
                        self.sb = 0
                        self.qt = 0
                        self.idx = 0
                        self.pexps = None

                    def done(self):
                        return self.sb >= SB

                    def step(self):
                        if self.done():
                            return
                        pr, sb, qt = self.pr, self.sb, self.qt
                        if qt == 0:
                            self.pexps = []
                            for hh in range(2):
                                # col 0 pad keeps cast chunks 4B-aligned
                                # (2x DVE/ACT modes); col 1 is P's zero col
                                pexp = wrk.tile([128, s + 2], BF16,
                                                tag=f"pexp{hh}", name="pexp")
                                nc.vector.memset(pexp[:, 1:2], 0.0)
                                self.pexps.append(pexp)
                        c0 = qt * CK
                        pss = []
                        for hh in range(2):
                            po = hh * 64
                            ps = ps_qa.tile([128, CK], F32, tag="qa_av",
                                            name="psA")
                            nc.tensor.matmul(
                                ps[:],
                                qT[pr][po:po + 64, sb * 128:(sb + 1) * 128],
                                erT[po:po + 64, c0:c0 + CK],
                                start=True, stop=True)
                            pss.append(ps)
                        for hh in range(2):
                            # casts mostly on DVE (ACT is exp-bound)
                            if self.idx % 4 == 3:
                                nc.scalar.copy(
                                    self.pexps[hh][:, 2 + c0:2 + c0 + CK],
                                    pss[hh][:])
                            else:
                                nc.vector.tensor_copy(
                                    self.pexps[hh][:, 2 + c0:2 + c0 + CK],
                                    pss[hh][:])
                            self.idx += 1
                        self.qt += 1
                        if self.qt == NCK:
                            for hh in range(2):
                                h = 2 * pr + hh
                                # SWDGE casts bf16 -> fp8 in flight,
                                # halving the bounce's HBM traffic
                                nc.gpsimd.dma_start(
                                    p_d[h, sb * 128 * (s + 1):
                                        (sb * 128 + 128) * (s + 1)]
                                    .rearrange("(r c) -> r c", c=s + 1),
                                    self.pexps[hh][:, 1:s + 2])
                            self.qt = 0
                            self.sb += 1

                def issue_srel_read(pr, qc, hh, qbi, eng):
                    """Skewed Srel row read. Issued on the gpsimd SWDGE
                    ring while phase A owns the Sync ring (pair 0's loop),
                    and on the idle Sync ring during pair 1 (the gpsimd
                    ring is then busy with collective triggers)."""
                    h = 2 * pr + hh
                    qb = qc * QBC + qbi
                    srt = srl.tile([128, s], F8, tag="srel", name="srel")
                    base = (qb * 128 + 1) * s
                    eng.dma_start(
                        srt[:],
                        p_d[h, base:base + 128 * s]
                        .rearrange("(r c) -> r c", c=s))
                    return srt

                def score_block(pr, qc, tb, srel_rows, expTs):
                    """scoresT[t-block tb, q-chunk qc] for both heads of the
                    pair in one 2-bank PSUM tile, then a single wide exp."""
                    q0 = qc * QPC
                    psT = ps_sc.tile([128, 2 * QPC], F32, tag="sc",
                                     name="psT")
                    for hh in range(2):
                        po = hh * 64
                        nc.tensor.matmul(
                            psT[:, hh * QPC:(hh + 1) * QPC],
                            kT[pr][po:po + 64, tb * 128:(tb + 1) * 128],
                            qT[pr][po:po + 64, q0:q0 + QPC],
                            start=True, stop=False)
                    for hh in range(2):
                        for qbi in range(QBC):
                            c0 = hh * QPC + qbi * 128
                            nc.tensor.matmul(
                                psT[:, c0:c0 + 128],
                                srel_rows[hh][qbi][:,
                                                   tb * 128:(tb + 1) * 128],
                                ident_f8[:],
                                start=False, stop=(qbi == QBC - 1))
                    et = expp.tile([128, 2 * QPC], BF16, tag="expT",
                                   name="expT")
                    nc.scalar.activation(
                        et[:], psT[:],
                        mybir.ActivationFunctionType.Exp, scale=SCALE)
                    expTs.append(et)

                def av_block(pr, qc, qbi, expTs, hh):
                    h = 2 * pr + hh
                    sb = qc * QBC + qbi
                    pc = ps_av.tile([128, HD + 1], F32, tag="qa_av",
                                    name="pc_av")
                    for tb in range(NTB):
                        nc.tensor.matmul(
                            pc[:],
                            expTs[tb][:, hh * QPC + qbi * 128:
                                      hh * QPC + (qbi + 1) * 128],
                            vaug[tb][:, h * (HD + 1):(h + 1) * (HD + 1)],
                            start=(tb == 0), stop=(tb == NTB - 1))
                    rinv = small.tile([128, 1], F32, tag="rinv", name="rinv")
                    nc.vector.reciprocal(rinv[:], pc[:, HD:HD + 1])
                    nc.vector.tensor_scalar_mul(
                        outp[sb][:, h * HD:(h + 1) * HD], pc[:, 0:HD],
                        rinv[:])

                def ln_stats_block(sb):
                    """(sum, sumsq) for one finished q block, all on DVE."""
                    ci = chunk_of[sb]
                    bi = sb - chunks[ci][0]
                    s1 = small.tile([128, 1], F32, tag="s1", name="s1")
                    nc.vector.reduce_sum(
                        s1[:], outp[sb][:], axis=mybir.AxisListType.X)
                    scr = lnw.tile([128, CH], F32, tag="scr", name="scr")
                    nc.vector.tensor_mul(scr[:], outp[sb][:], outp[sb][:])
                    sq = small.tile([128, 1], F32, tag="sq", name="sq")
                    nc.vector.reduce_sum(
                        sq[:], scr[:], axis=mybir.AxisListType.X)
                    # scalar HWDGE ring: the sync ring carries the bulky
                    # srel reads in pair 1, which would delay the stats and
                    # thus the collective triggers
                    nc.scalar.dma_start(
                        cc_in_q[ci][bi * 128:(bi + 1) * 128, 0:1], s1[:])
                    nc.scalar.dma_start(
                        cc_in_q[ci][bi * 128:(bi + 1) * 128, 1:2], sq[:])
                    if sb == chunks[ci][1] - 1:
                        ln_collective(ci)
                        pending_out.append(ci)

                def ln_collective(ci):
                    """AllReduce this chunk's (sum, sumsq) with the peer."""
                    nc.gpsimd.collective_compute(
                        "AllReduce", mybir.AluOpType.add,
                        replica_groups=pairs,
                        ins=[cc_in_q[ci][:].opt()],
                        outs=[cc_out_q[ci][:].opt()])

                def ln_output(ci):
                    """Final LayerNorm for one chunk (after its AllReduce).
                    Deferred into the next chunk's score loop so the DVE
                    FIFO never blocks the AV epilogue on a collective."""
                    for bi in range(chunks[ci][1] - chunks[ci][0]):
                        sb = chunks[ci][0] + bi
                        st = small.tile([128, 2], F32, tag="st")
                        nc.sync.dma_start(
                            st[:], cc_out_q[ci][bi * 128:(bi + 1) * 128, :])
                        mean = small.tile([128, 1], F32, tag="mean")
                        nc.vector.tensor_scalar_mul(
                            mean[:], st[:, 0:1], 1.0 / E)
                        ex2 = small.tile([128, 1], F32, tag="ex2")
                        nc.vector.tensor_scalar_mul(
                            ex2[:], st[:, 1:2], 1.0 / E)
                        msq = small.tile([128, 1], F32, tag="msq")
                        nc.vector.tensor_mul(msq[:], mean[:], mean[:])
                        var = small.tile([128, 1], F32, tag="var")
                        nc.vector.tensor_sub(var[:], ex2[:], msq[:])
                        std = small.tile([128, 1], F32, tag="std")
                        nc.scalar.activation(
                            std[:], var[:],
                            mybir.ActivationFunctionType.Sqrt, bias=eps_t[:])
                        rstd = small.tile([128, 1], F32, tag="rstd")
                        nc.vector.reciprocal(rstd[:], std[:])
                        tmp = lnw.tile([128, CH], F32, tag="tmp")
                        nc.vector.tensor_scalar(
                            tmp[:], outp[sb][:], mean[:], rstd[:],
                            op0=mybir.AluOpType.subtract,
                            op1=mybir.AluOpType.mult)
                        y1 = lnw.tile([128, CH], F32, tag="y1")
                        nc.vector.tensor_mul(y1[:], tmp[:], gamma_bc[:])
                        y2 = lnw.tile([128, CH], F32, tag="y2")
                        nc.vector.tensor_add(y2[:], y1[:], beta_bc[:])
                        nc.sync.dma_start(
                            out_d[sb * 128:(sb + 1) * 128, :], y2[:])

                # warm up the CC stream off the critical path: the first
                # collective pays a large one-time cost (~40us observed)
                zt = small.tile([128, 2], F32, tag="zt", name="zt")
                nc.vector.memset(zt[:], 0.0)
                nc.sync.dma_start(cc_in_q[0][0:128, :], zt[:])
                nc.gpsimd.collective_compute(
                    "AllReduce", mybir.AluOpType.add,
                    replica_groups=pairs,
                    ins=[cc_in_q[0][0:128, :].opt()],
                    outs=[cc_out_q[0][0:128, :].opt()])

                # ---- main schedule ----
                steps = [(pr, qc) for pr in range(2) for qc in range(NQC)]

                # prologue: P rows for pair 0 (pair 1's interleave below)
                pa1 = PhaseA(1)
                pa0 = PhaseA(0)
                while not pa0.done():
                    pa0.step()

                srel_cache = {
                    steps[0]: [[issue_srel_read(0, 0, hh, qbi, nc.gpsimd)
                                for qbi in range(QBC)] for hh in range(2)]
                }

                # prefetch rate: srel reads for the next iteration per tb
                pending_out = []
                rate = (2 * QBC + NTB - 1) // NTB
                for si, (pr, qc) in enumerate(steps):
                    srel_rows = srel_cache.pop((pr, qc))
                    nxt = steps[si + 1] if si + 1 < len(steps) else None
                    expTs = []
                    for tb in range(NTB):
                        score_block(pr, qc, tb, srel_rows, expTs)
                        if pr == 0:
                            pa1.step()
                        elif tb == NTB // 2:
                            while pending_out:
                                ln_output(pending_out.pop(0))
                        if nxt is not None:
                            for j in range(rate):
                                ri = tb * rate + j
                                if ri < 2 * QBC:
                                    if ri == 0:
                                        srel_cache[nxt] = [[], []]
                                    hh2, qbi2 = ri // QBC, ri % QBC
                                    if nxt[0] == 1:
                                        # the skew read of q-block qb also
                                        # touches the first P row of block
                                        # qb+1: make sure pair 1's phase A
                                        # has emitted those writes
                                        need = min(
                                            nxt[1] * QBC + qbi2 + 2, SB)
                                        while (not pa1.done()
                                               and pa1.sb < need):
                                            pa1.step()
                                    srel_cache[nxt][hh2].append(
                                        issue_srel_read(
                                            nxt[0], nxt[1], hh2, qbi2,
                                            nc.gpsimd if pr == 0
                                            else nc.scalar))
                    qbis = list(range(QBC))
                    if si + 1 == len(steps) and QBC > 1:
                        qbis = [QBC - 1] + qbis[:-1]
                    for qbi in qbis:
                        for hh in range(2):
                            av_block(pr, qc, qbi, expTs, hh)
                        if pr == 1:
                            ln_stats_block(qc * QBC + qbi)
                    if pr == 1 and si + 1 == len(steps):
                        while pending_out:
                            ln_output(pending_out.pop(0))

    if legalize:
        _legalize_waits(nc)
    return nc


def _legalize_waits(nc):
    """walrus's codegen accepts at most one sync wait on most instruction
    structs; hoist extra waits onto NoOps inserted just before, on the
    same engine queue (program order preserves the semantics)."""
    n = 0
    keep = set()
    for bb in nc.main_func.blocks:
        out = []
        for inst in bb.instructions:
            si = inst.sync_info
            if (inst.opcode not in keep and si is not None
                    and si.on_wait and len(si.on_wait) > 1):
                for w in si.on_wait[:-1]:
                    nop = mybir.InstNoOp(
                        name=f"I-mmw{n}", ins=[], outs=[])
                    n += 1
                    nop.engine = inst.engine
                    nop.sync_info = mybir.SyncInfo(
                        on_wait=[w], on_update=[])
                    out.append(nop)
                si.on_wait = [si.on_wait[-1]]
            out.append(inst)
        bb.instructions = out
    return nc


_NC_CACHE = {}


def _get_nc(s=S, n_cores=N_CORES):
    key = (s, n_cores)
    if key not in _NC_CACHE:
        _NC_CACHE[key] = build_nc(s, n_cores)
    return _NC_CACHE[key]


def make_in_maps(x, Wq, Wk, Wv, Er, gamma, beta, n_cores=N_CORES):
    in_maps = []
    for c in range(n_cores):
        b, hg = c // 2, c % 2
        sl = slice(hg * CH, (hg + 1) * CH)
        in_maps.append({
            "x": np.ascontiguousarray(x[b], dtype=np.float32),
            "wq": np.ascontiguousarray(Wq[sl], dtype=np.float32),
            "wk": np.ascontiguousarray(Wk[sl], dtype=np.float32),
            "wv": np.ascontiguousarray(Wv[sl], dtype=np.float32),
            "er": np.ascontiguousarray(Er, dtype=np.float32),
            "gamma": np.ascontiguousarray(gamma[sl], dtype=np.float32)[None, :],
            "beta": np.ascontiguousarray(beta[sl], dtype=np.float32)[None, :],
        })
    return in_maps


def assemble(results, n_cores=N_CORES, s=S):
    y = np.empty((n_cores // 2, s, E), np.float32)
    for c in range(n_cores):
        y[c // 2, :, (c % 2) * CH:(c % 2 + 1) * CH] = results[c]["out"]
    return y


def kernel(**inputs):
    from concourse.bass_utils import run_bass_kernel_spmd
    nc = _get_nc()
    in_maps = make_in_maps(
        inputs["x"], inputs["Wq"], inputs["Wk"], inputs["Wv"],
        inputs["Er"], inputs["gamma"], inputs["beta"])
    res = run_bass_kernel_spmd(nc, in_maps, list(range(N_CORES)))
    return assemble(res.results)
